# revision 1
# baseline (speedup 1.0000x reference)
import sys

sys.path.insert(0, "/opt/trn_rl_repo")
import numpy as np

_T = 0.1
_BT = 0.07
_EPS = 1e-10
_NCORES = 8
_A = 64        # anchors
_V = 128       # views per anchor
_D = 256       # feature dim
_N = _A * _V   # 8192 rows
_TILES = 8     # row tiles (128 rows) per core
_SS = 4        # supersteps per tile
_SSW = 2048    # superstep width (4 PSUM banks)
_CW = 512      # matmul moving width (f32)

PROFILE = False
LAST_EXEC_NS = None
_cache = {}


def _build(labels):
    from concourse import bass, bacc, mybir, tile

    F32 = mybir.dt.float32

    aperm = np.argsort(labels, kind="stable")
    slab = labels[aperm]
    first = np.searchsorted(slab, slab, side="left")
    last = np.searchsorted(slab, slab, side="right")
    OFF, WID = [], []
    for t in range(_TILES):
        lo = int(first[8 * t]) * _V
        hi = int(last[8 * t + 7]) * _V
        OFF.append(lo)
        WID.append(hi - lo)
    Wmax = max(WID)

    nc = bacc.Bacc()
    cft_d = nc.declare_dram_parameter("cft", [_D, _N], F32, isOutput=False)
    rwt_d = nc.declare_dram_parameter("rwt", [_V, 2 * _TILES * _V], F32, isOutput=False)
    mng_d = nc.declare_dram_parameter("mng", [_V, _TILES], F32, isOutput=False)
    out_d = nc.declare_dram_parameter("outv", [_TILES * _V, 3], F32, isOutput=True)

    with tile.TileContext(nc) as tc:
        with (
            tc.tile_pool(name="persist", bufs=1) as pp,
            tc.tile_pool(name="epool", bufs=2) as ep,
            tc.tile_pool(name="small", bufs=2) as sp,
            tc.tile_pool(name="lnp", bufs=(2 if Wmax <= 4096 else 1)) as lp,
            tc.tile_pool(name="psum", bufs=2, space=bass.MemorySpace.PSUM) as qp,
        ):
            cft0 = pp.tile([_V, _N], F32)
            cft1 = pp.tile([_V, _N], F32)
            rwt = pp.tile([_V, 2 * _TILES * _V], F32)
            mng = pp.tile([_V, _TILES], F32)
            cfts = [cft0, cft1]

            nc.sync.dma_start(rwt[:], rwt_d[:])
            nc.sync.dma_start(mng[:], mng_d[:])
            for p in range(_SS):
                for kc in range(2):
                    nc.sync.dma_start(
                        cfts[kc][:, p * _SSW : (p + 1) * _SSW],
                        cft_d[kc * _V : (kc + 1) * _V, p * _SSW : (p + 1) * _SSW],
                    )

            for t in range(_TILES):
                et = ep.tile([_V, _N], F32)
                esp = sp.tile([_V, _SS], F32)
                outt = sp.tile([_V, 4], F32)
                zt = sp.tile([_V, 1], F32)
                zbt = sp.tile([_V, 1], F32)
                for ss in range(_SS):
                    acc = qp.tile([_V, _SSW], F32)
                    for kc in range(2):
                        lhsT = rwt[:, (2 * t + kc) * _V : (2 * t + kc + 1) * _V]
                        for cc in range(_SSW // _CW):
                            c0 = ss * _SSW + cc * _CW
                            nc.tensor.matmul(
                                acc[:, cc * _CW : (cc + 1) * _CW],
                                lhsT,
                                cfts[kc][:, c0 : c0 + _CW],
                                start=(kc == 0),
                                stop=(kc == 1),
                            )
                    nc.scalar.activation(
                        et[:, ss * _SSW : (ss + 1) * _SSW],
                        acc[:],
                        mybir.ActivationFunctionType.Exp,
                        bias=mng[:, t : t + 1],
                        scale=1.0 / _T,
                        accum_out=esp[:, ss : ss + 1],
                    )
                nc.vector.tensor_reduce(
                    outt[:, 0:1], esp[:, 0:_SS],
                    mybir.AxisListType.X, mybir.AluOpType.add,
                )
                nc.vector.tensor_reduce(
                    outt[:, 1:2], et[:, OFF[t] : OFF[t] + WID[t]],
                    mybir.AxisListType.X, mybir.AluOpType.add,
                )
                nc.vector.tensor_scalar(
                    zt[:], outt[:, 0:1], outt[:, 1:2], None, mybir.AluOpType.subtract
                )
                nc.vector.tensor_scalar(
                    zbt[:], zt[:], 0.0, _EPS, mybir.AluOpType.max, mybir.AluOpType.add
                )
                lns = lp.tile([_V, Wmax], F32)
                nc.scalar.activation(
                    lns[:, 0 : WID[t]],
                    et[:, OFF[t] : OFF[t] + WID[t]],
                    mybir.ActivationFunctionType.Ln,
                    bias=zbt[:],
                    accum_out=outt[:, 2:3],
                )
                nc.sync.dma_start(out_d[t * _V : (t + 1) * _V, 0:3], outt[:, 0:3])

    nc.finalize()
    return dict(nc=nc, aperm=aperm, slab=slab, OFF=OFF, WID=WID,
                first=first, last=last)


def kernel(feats_, labels_):
    global LAST_EXEC_NS
    from concourse.bass_utils import run_bass_kernel_spmd

    labels = np.asarray(labels_, dtype=np.int32)
    feats = np.asarray(feats_, dtype=np.float32)
    key = labels.tobytes()
    if key not in _cache:
        _cache[key] = _build(labels)
    pr = _cache[key]
    aperm, slab = pr["aperm"], pr["slab"]
    OFF, WID = pr["OFF"], pr["WID"]

    G = feats[aperm].reshape(_N, _D)
    cft = np.ascontiguousarray(G.T)
    G64 = G.astype(np.float64)
    normsq = np.einsum("ij,ij->i", G64, G64)
    m = normsq / _T
    mneg32 = (-m).astype(np.float32)

    in_maps = []
    for k in range(_NCORES):
        rwt = np.empty((_V, 2 * _TILES * _V), np.float32)
        mng = np.empty((_V, _TILES), np.float32)
        for t in range(_TILES):
            rb = (8 * t + k) * _V
            blk = G[rb : rb + _V, :].T  # [256, 128]
            rwt[:, (2 * t) * _V : (2 * t + 1) * _V] = blk[:_V, :]
            rwt[:, (2 * t + 1) * _V : (2 * t + 2) * _V] = blk[_V:, :]
            mng[:, t] = mneg32[rb : rb + _V]
        in_maps.append({"cft": cft, "rwt": rwt, "mng": mng})

    res = run_bass_kernel_spmd(
        pr["nc"], in_maps, core_ids=list(range(_NCORES)), trace=PROFILE
    )
    LAST_EXEC_NS = res.exec_time_ns

    Esum = np.empty(_N, np.float32)
    Eblk = np.empty(_N, np.float32)
    Lblk = np.empty(_N, np.float32)
    for k in range(_NCORES):
        o = res.results[k]["outv"]
        for t in range(_TILES):
            g0 = (8 * t + k) * _V
            Esum[g0 : g0 + _V] = o[t * _V : (t + 1) * _V, 0]
            Eblk[g0 : g0 + _V] = o[t * _V : (t + 1) * _V, 1]
            Lblk[g0 : g0 + _V] = o[t * _V : (t + 1) * _V, 2]

    row_slot = np.arange(_N) // _V
    counts = (pr["last"] - pr["first"]).astype(np.int64)     # n_c per slot
    ncls_rows = (counts[row_slot] * _V).astype(np.float64)   # same-class rows
    Wrow = np.asarray(WID, np.float64)[row_slot // 8]
    nwrong = Wrow - ncls_rows

    z = Esum - Eblk                                   # f32
    zb = np.maximum(z, np.float32(0.0)) + np.float32(_EPS)  # f32, matches device
    diag_term = np.log(np.float32(1.0) + zb).astype(np.float64)
    wrong_term = nwrong * np.log(zb.astype(np.float64))
    Lpos = Lblk.astype(np.float64) - diag_term - wrong_term

    row_class = slab[row_slot]
    Csum_row = np.empty((_N, _D), np.float64)
    for c in np.unique(slab):
        sel = row_class == c
        Csum_row[sel] = G64[sel].sum(axis=0)
    Scross = np.einsum("ij,ij->i", G64, Csum_row)
    S = (Scross - normsq) / _T - (ncls_rows - 1.0) * m
    P = ncls_rows - 1.0

    mlpp = (S - Lpos) / P
    loss = np.mean(-(_T / _BT) * mlpp)
    return np.array(loss, dtype=np.float32)


# revision 5
# speedup vs baseline: 1.8300x; 1.8300x over previous
import sys

sys.path.insert(0, "/opt/trn_rl_repo")
import numpy as np

_T = 0.1
_BT = 0.07
_EPS = 1e-10
_NCORES = 8
_A = 64        # anchors
_V = 128       # views per anchor
_D = 256       # feature dim
_N = _A * _V   # 8192 rows
_TILES = 8     # row tiles (128 rows) per core
_SS = 4        # supersteps per tile
_SSW = 2048    # superstep width (4 PSUM banks)
_CW = 512      # matmul moving width (f32)

PROFILE = False
LAST_EXEC_NS = None
_cache = {}


def _build(labels):
    from concourse import bass, bacc, mybir, tile

    F32 = mybir.dt.float32
    BF16 = mybir.dt.bfloat16

    aperm = np.argsort(labels, kind="stable")
    slab = labels[aperm]
    first = np.searchsorted(slab, slab, side="left")
    last = np.searchsorted(slab, slab, side="right")
    OFF, WID = [], []
    for t in range(_TILES):
        lo = int(first[8 * t]) * _V
        hi = int(last[8 * t + 7]) * _V
        OFF.append(lo)
        WID.append(hi - lo)
    Wmax = max(WID)

    nc = bacc.Bacc()
    cft_d = nc.declare_dram_parameter("cft", [_D, _N], BF16, isOutput=False)
    rwt_d = nc.declare_dram_parameter("rwt", [_V, 2 * _TILES * _V], BF16, isOutput=False)
    mng_d = nc.declare_dram_parameter("mng", [_V, _TILES], F32, isOutput=False)
    out_d = nc.declare_dram_parameter("outv", [_TILES * _V, 3], F32, isOutput=True)

    with tile.TileContext(nc) as tc:
        with (
            tc.tile_pool(name="persist", bufs=1) as pp,
            tc.tile_pool(name="epool", bufs=2) as ep,
            tc.tile_pool(name="small", bufs=2) as sp,
            tc.tile_pool(name="lnp", bufs=(2 if Wmax <= 4096 else 1)) as lp,
            tc.tile_pool(name="psum", bufs=2, space=bass.MemorySpace.PSUM) as qp,
        ):
            cft0 = pp.tile([_V, _N], BF16)
            cft1 = pp.tile([_V, _N], BF16)
            rwt = pp.tile([_V, 2 * _TILES * _V], BF16)
            mng = pp.tile([_V, _TILES], F32)
            cfts = [cft0, cft1]

            nc.sync.dma_start(rwt[:], rwt_d[:])
            nc.sync.dma_start(mng[:], mng_d[:])
            for p in range(_SS):
                for kc in range(2):
                    nc.sync.dma_start(
                        cfts[kc][:, p * _SSW : (p + 1) * _SSW],
                        cft_d[kc * _V : (kc + 1) * _V, p * _SSW : (p + 1) * _SSW],
                    )

            for t in range(_TILES):
                et = ep.tile([_V, _N], F32)
                esp = sp.tile([_V, _SS], F32)
                outt = sp.tile([_V, 4], F32)
                zt = sp.tile([_V, 1], F32)
                zbt = sp.tile([_V, 1], F32)
                for ss in range(_SS):
                    acc = qp.tile([_V, _SSW], F32)
                    for kc in range(2):
                        lhsT = rwt[:, (2 * t + kc) * _V : (2 * t + kc + 1) * _V]
                        for cc in range(_SSW // _CW):
                            c0 = ss * _SSW + cc * _CW
                            nc.tensor.matmul(
                                acc[:, cc * _CW : (cc + 1) * _CW],
                                lhsT,
                                cfts[kc][:, c0 : c0 + _CW],
                                start=(kc == 0),
                                stop=(kc == 1),
                            )
                    nc.scalar.activation(
                        et[:, ss * _SSW : (ss + 1) * _SSW],
                        acc[:],
                        mybir.ActivationFunctionType.Exp,
                        bias=mng[:, t : t + 1],
                        scale=1.0 / _T,
                        accum_out=esp[:, ss : ss + 1],
                    )
                nc.vector.tensor_reduce(
                    outt[:, 0:1], esp[:, 0:_SS],
                    mybir.AxisListType.X, mybir.AluOpType.add,
                )
                nc.vector.tensor_reduce(
                    outt[:, 1:2], et[:, OFF[t] : OFF[t] + WID[t]],
                    mybir.AxisListType.X, mybir.AluOpType.add,
                )
                nc.vector.tensor_scalar(
                    zt[:], outt[:, 0:1], outt[:, 1:2], None, mybir.AluOpType.subtract
                )
                nc.vector.tensor_scalar(
                    zbt[:], zt[:], 0.0, _EPS, mybir.AluOpType.max, mybir.AluOpType.add
                )
                lns = lp.tile([_V, Wmax], F32)
                nc.scalar.activation(
                    lns[:, 0 : WID[t]],
                    et[:, OFF[t] : OFF[t] + WID[t]],
                    mybir.ActivationFunctionType.Ln,
                    bias=zbt[:],
                    accum_out=outt[:, 2:3],
                )
                nc.sync.dma_start(out_d[t * _V : (t + 1) * _V, 0:3], outt[:, 0:3])

    nc.finalize()
    return dict(nc=nc, aperm=aperm, slab=slab, OFF=OFF, WID=WID,
                first=first, last=last)


def kernel(feats_, labels_):
    global LAST_EXEC_NS
    from concourse.bass_utils import run_bass_kernel_spmd

    labels = np.asarray(labels_, dtype=np.int32)
    feats = np.asarray(feats_, dtype=np.float32)
    key = labels.tobytes()
    if key not in _cache:
        _cache[key] = _build(labels)
    pr = _cache[key]
    aperm, slab = pr["aperm"], pr["slab"]
    OFF, WID = pr["OFF"], pr["WID"]

    import ml_dtypes

    BF = ml_dtypes.bfloat16
    G = feats[aperm].reshape(_N, _D)
    G16 = G.astype(BF)
    cft = np.ascontiguousarray(G16.T)
    G64 = G.astype(np.float64)
    normsq = np.einsum("ij,ij->i", G64, G64)
    m = normsq / _T
    G16f = G16.astype(np.float64)
    m16 = np.einsum("ij,ij->i", G16f, G16f) / _T
    mneg32 = (-m16).astype(np.float32)

    in_maps = []
    for k in range(_NCORES):
        rwt = np.empty((_V, 2 * _TILES * _V), BF)
        mng = np.empty((_V, _TILES), np.float32)
        for t in range(_TILES):
            rb = (8 * t + k) * _V
            blk = G16[rb : rb + _V, :].T  # [256, 128]
            rwt[:, (2 * t) * _V : (2 * t + 1) * _V] = blk[:_V, :]
            rwt[:, (2 * t + 1) * _V : (2 * t + 2) * _V] = blk[_V:, :]
            mng[:, t] = mneg32[rb : rb + _V]
        in_maps.append({"cft": cft, "rwt": rwt, "mng": mng})

    res = run_bass_kernel_spmd(
        pr["nc"], in_maps, core_ids=list(range(_NCORES)), trace=PROFILE
    )
    LAST_EXEC_NS = res.exec_time_ns

    Esum = np.empty(_N, np.float32)
    Eblk = np.empty(_N, np.float32)
    Lblk = np.empty(_N, np.float32)
    for k in range(_NCORES):
        o = res.results[k]["outv"]
        for t in range(_TILES):
            g0 = (8 * t + k) * _V
            Esum[g0 : g0 + _V] = o[t * _V : (t + 1) * _V, 0]
            Eblk[g0 : g0 + _V] = o[t * _V : (t + 1) * _V, 1]
            Lblk[g0 : g0 + _V] = o[t * _V : (t + 1) * _V, 2]

    row_slot = np.arange(_N) // _V
    counts = (pr["last"] - pr["first"]).astype(np.int64)     # n_c per slot
    ncls_rows = (counts[row_slot] * _V).astype(np.float64)   # same-class rows
    Wrow = np.asarray(WID, np.float64)[row_slot // 8]
    nwrong = Wrow - ncls_rows

    z = Esum - Eblk                                   # f32
    zb = np.maximum(z, np.float32(0.0)) + np.float32(_EPS)  # f32, matches device
    diag_term = np.log(np.float32(1.0) + zb).astype(np.float64)
    wrong_term = nwrong * np.log(zb.astype(np.float64))
    Lpos = Lblk.astype(np.float64) - diag_term - wrong_term

    row_class = slab[row_slot]
    Csum_row = np.empty((_N, _D), np.float64)
    for c in np.unique(slab):
        sel = row_class == c
        Csum_row[sel] = G64[sel].sum(axis=0)
    Scross = np.einsum("ij,ij->i", G64, Csum_row)
    S = (Scross - normsq) / _T - (ncls_rows - 1.0) * m
    P = ncls_rows - 1.0

    mlpp = (S - Lpos) / P
    loss = np.mean(-(_T / _BT) * mlpp)
    return np.array(loss, dtype=np.float32)


# revision 9
# speedup vs baseline: 1.8548x; 1.0136x over previous
import sys

sys.path.insert(0, "/opt/trn_rl_repo")
import numpy as np

_T = 0.1
_BT = 0.07
_EPS = 1e-10
_NCORES = 8
_A = 64        # anchors
_V = 128       # views per anchor
_D = 256       # feature dim
_N = _A * _V   # 8192 rows
_TILES = 8     # row tiles (128 rows) per core
_SS = 4        # supersteps per tile
_SSW = 2048    # superstep width (4 PSUM banks)
_CW = 512      # matmul moving width

PROFILE = False
LAST_EXEC_NS = None
_cache = {}


def _build(labels):
    from concourse import bass, bacc, mybir, tile

    F32 = mybir.dt.float32
    BF16 = mybir.dt.bfloat16

    aperm = np.argsort(labels, kind="stable")
    slab = labels[aperm]
    first = np.searchsorted(slab, slab, side="left")
    last = np.searchsorted(slab, slab, side="right")
    OFF, WID = [], []
    for t in range(_TILES):
        lo = int(first[8 * t]) * _V
        hi = int(last[8 * t + 7]) * _V
        OFF.append(lo)
        WID.append(hi - lo)
    Wmax = max(WID)
    defer = Wmax <= 4096  # deferred-Ln needs wtile [V, 8*Wmax] bf16 in SBUF

    nc = bacc.Bacc()
    cft_d = nc.declare_dram_parameter("cft", [_D, _N], BF16, isOutput=False)
    rwt_d = nc.declare_dram_parameter("rwt", [_V, 2 * _TILES * _V], BF16, isOutput=False)
    mng_d = nc.declare_dram_parameter("mng", [_V, _TILES], F32, isOutput=False)
    out_d = nc.declare_dram_parameter("outv", [_TILES * _V, 2], F32, isOutput=True)

    with tile.TileContext(nc) as tc:
        with (
            tc.tile_pool(name="persist", bufs=1) as pp,
            tc.tile_pool(name="epool", bufs=2) as ep,
            tc.tile_pool(name="small", bufs=2) as sp,
            tc.tile_pool(name="lnp", bufs=2) as lp,
            tc.tile_pool(name="psum", bufs=2, space=bass.MemorySpace.PSUM) as qp,
        ):
            cft0 = pp.tile([_V, _N], BF16)
            cft1 = pp.tile([_V, _N], BF16)
            rwt = pp.tile([_V, 2 * _TILES * _V], BF16)
            mng = pp.tile([_V, _TILES], F32)
            zbs = pp.tile([_V, _TILES], F32)
            wtile = (
                pp.tile([_V, _TILES * Wmax], BF16, name="wtile") if defer else None
            )
            cfts = [cft0, cft1]

            nc.sync.dma_start(rwt[:], rwt_d[:])
            nc.sync.dma_start(mng[:], mng_d[:])
            for p in range(_SS):
                for kc in range(2):
                    nc.sync.dma_start(
                        cfts[kc][:, p * _SSW : (p + 1) * _SSW],
                        cft_d[kc * _V : (kc + 1) * _V, p * _SSW : (p + 1) * _SSW],
                    )

            for t in range(_TILES):
                et = ep.tile([_V, _N], BF16)
                es = sp.tile([_V, 1], F32)
                eb = sp.tile([_V, 1], F32)
                zt = sp.tile([_V, 1], F32)
                for ss in range(_SS):
                    acc = qp.tile([_V, _SSW], F32)
                    for kc in range(2):
                        lhsT = rwt[:, (2 * t + kc) * _V : (2 * t + kc + 1) * _V]
                        for cc in range(_SSW // _CW):
                            c0 = ss * _SSW + cc * _CW
                            nc.tensor.matmul(
                                acc[:, cc * _CW : (cc + 1) * _CW],
                                lhsT,
                                cfts[kc][:, c0 : c0 + _CW],
                                start=(kc == 0),
                                stop=(kc == 1),
                            )
                    nc.scalar.activation(
                        et[:, ss * _SSW : (ss + 1) * _SSW],
                        acc[:],
                        mybir.ActivationFunctionType.Exp,
                        bias=mng[:, t : t + 1],
                        scale=1.0 / _T,
                    )
                nc.vector.tensor_reduce(
                    es[:], et[:, 0:_N], mybir.AxisListType.X, mybir.AluOpType.add
                )
                nc.vector.tensor_reduce(
                    eb[:], et[:, OFF[t] : OFF[t] + WID[t]],
                    mybir.AxisListType.X, mybir.AluOpType.add,
                )
                nc.vector.tensor_scalar(
                    zt[:], es[:], eb[:], None, mybir.AluOpType.subtract
                )
                nc.vector.tensor_scalar(
                    zbs[:, t : t + 1], zt[:], 0.0, _EPS,
                    mybir.AluOpType.max, mybir.AluOpType.add,
                )
                nc.sync.dma_start(out_d[t * _V : (t + 1) * _V, 0:1], zbs[:, t : t + 1])
                if defer:
                    nc.sync.dma_start(
                        wtile[:, t * Wmax : t * Wmax + WID[t]],
                        et[:, OFF[t] : OFF[t] + WID[t]],
                    )
                else:
                    lns = lp.tile([_V, Wmax], BF16)
                    lb = sp.tile([_V, 1], F32)
                    nc.scalar.activation(
                        lns[:, 0 : WID[t]],
                        et[:, OFF[t] : OFF[t] + WID[t]],
                        mybir.ActivationFunctionType.Ln,
                        bias=zbs[:, t : t + 1],
                        accum_out=lb[:],
                    )
                    nc.sync.dma_start(out_d[t * _V : (t + 1) * _V, 1:2], lb[:])

            if defer:
                for t in range(_TILES):
                    lns = lp.tile([_V, Wmax], BF16)
                    lb = sp.tile([_V, 1], F32)
                    nc.scalar.activation(
                        lns[:, 0 : WID[t]],
                        wtile[:, t * Wmax : t * Wmax + WID[t]],
                        mybir.ActivationFunctionType.Ln,
                        bias=zbs[:, t : t + 1],
                        accum_out=lb[:],
                    )
                    nc.sync.dma_start(out_d[t * _V : (t + 1) * _V, 1:2], lb[:])

    nc.finalize()
    return dict(nc=nc, aperm=aperm, slab=slab, OFF=OFF, WID=WID,
                first=first, last=last)


def kernel(feats_, labels_):
    global LAST_EXEC_NS
    from concourse.bass_utils import run_bass_kernel_spmd

    labels = np.asarray(labels_, dtype=np.int32)
    feats = np.asarray(feats_, dtype=np.float32)
    key = labels.tobytes()
    if key not in _cache:
        _cache[key] = _build(labels)
    pr = _cache[key]
    aperm, slab = pr["aperm"], pr["slab"]
    OFF, WID = pr["OFF"], pr["WID"]

    import ml_dtypes

    BF = ml_dtypes.bfloat16
    G = feats[aperm].reshape(_N, _D)
    G16 = G.astype(BF)
    cft = np.ascontiguousarray(G16.T)
    G64 = G.astype(np.float64)
    normsq = np.einsum("ij,ij->i", G64, G64)
    m = normsq / _T
    G16f = G16.astype(np.float64)
    m16 = np.einsum("ij,ij->i", G16f, G16f) / _T
    mneg32 = (-m16).astype(np.float32)

    in_maps = []
    for k in range(_NCORES):
        rwt = np.empty((_V, 2 * _TILES * _V), BF)
        mng = np.empty((_V, _TILES), np.float32)
        for t in range(_TILES):
            rb = (8 * t + k) * _V
            blk = G16[rb : rb + _V, :].T  # [256, 128]
            rwt[:, (2 * t) * _V : (2 * t + 1) * _V] = blk[:_V, :]
            rwt[:, (2 * t + 1) * _V : (2 * t + 2) * _V] = blk[_V:, :]
            mng[:, t] = mneg32[rb : rb + _V]
        in_maps.append({"cft": cft, "rwt": rwt, "mng": mng})

    res = run_bass_kernel_spmd(
        pr["nc"], in_maps, core_ids=list(range(_NCORES)), trace=PROFILE
    )
    LAST_EXEC_NS = res.exec_time_ns

    zb = np.empty(_N, np.float32)
    Lblk = np.empty(_N, np.float32)
    for k in range(_NCORES):
        o = res.results[k]["outv"]
        for t in range(_TILES):
            g0 = (8 * t + k) * _V
            zb[g0 : g0 + _V] = o[t * _V : (t + 1) * _V, 0]
            Lblk[g0 : g0 + _V] = o[t * _V : (t + 1) * _V, 1]

    row_slot = np.arange(_N) // _V
    counts = (pr["last"] - pr["first"]).astype(np.int64)     # n_c per slot
    ncls_rows = (counts[row_slot] * _V).astype(np.float64)   # same-class rows
    Wrow = np.asarray(WID, np.float64)[row_slot // 8]
    nwrong = Wrow - ncls_rows

    diag_term = np.log(np.float32(1.0) + zb).astype(np.float64)
    wrong_term = nwrong * np.log(zb.astype(np.float64))
    Lpos = Lblk.astype(np.float64) - diag_term - wrong_term

    row_class = slab[row_slot]
    Csum_row = np.empty((_N, _D), np.float64)
    for c in np.unique(slab):
        sel = row_class == c
        Csum_row[sel] = G64[sel].sum(axis=0)
    Scross = np.einsum("ij,ij->i", G64, Csum_row)
    S = (Scross - normsq) / _T - (ncls_rows - 1.0) * m
    P = ncls_rows - 1.0

    mlpp = (S - Lpos) / P
    loss = np.mean(-(_T / _BT) * mlpp)
    return np.array(loss, dtype=np.float32)


# revision 15
# speedup vs baseline: 2.2202x; 1.1970x over previous
import sys

sys.path.insert(0, "/opt/trn_rl_repo")
import numpy as np

_T = 0.1
_BT = 0.07
_EPS = 1e-10
_NCORES = 8
_A = 64        # anchors
_V = 128       # views per anchor
_D = 256       # feature dim
_N = _A * _V   # 8192 rows
_TILES = 8     # row tiles (128 rows) per core
_SS = 4        # supersteps per tile
_SSW = 2048    # superstep width (4 PSUM banks)
_CW = 512      # matmul moving width

PROFILE = False
LAST_EXEC_NS = None
_cache = {}


def _patch_act_tables():
    from concourse import bacc

    if getattr(bacc, "_joint_act_patch", False):
        return
    orig = bacc.get_activation_tables

    def _joint_only(arch):
        tabs = orig(arch)
        return {
            n: (s if n == "natural_log_exp_and_others" else set())
            for n, s in tabs.items()
        }

    bacc.get_activation_tables = _joint_only
    bacc._joint_act_patch = True


def _build(labels):
    from concourse import bass, bacc, mybir, tile

    _patch_act_tables()
    F32 = mybir.dt.float32
    BF16 = mybir.dt.bfloat16

    aperm = np.argsort(labels, kind="stable")
    slab = labels[aperm]
    first = np.searchsorted(slab, slab, side="left")
    last = np.searchsorted(slab, slab, side="right")
    OFF, WID = [], []
    for t in range(_TILES):
        lo = int(first[8 * t]) * _V
        hi = int(last[8 * t + 7]) * _V
        OFF.append(lo)
        WID.append(hi - lo)

    nc = bacc.Bacc()
    cft_d = nc.declare_dram_parameter("cft", [_D, _N], BF16, isOutput=False)
    rwt_d = nc.declare_dram_parameter("rwt", [_V, 2 * _TILES * _V], BF16, isOutput=False)
    mng_d = nc.declare_dram_parameter("mng", [_V, _TILES], F32, isOutput=False)
    out_d = nc.declare_dram_parameter("outv", [_TILES * _V, 2], F32, isOutput=True)

    with tile.TileContext(nc) as tc:
        with (
            tc.tile_pool(name="persist", bufs=1) as pp,
            tc.tile_pool(name="epool", bufs=3) as ep,
            tc.tile_pool(name="small", bufs=2) as sp,
            tc.tile_pool(name="lnp", bufs=2) as lp,
            tc.tile_pool(name="psum", bufs=2, space=bass.MemorySpace.PSUM) as qp,
        ):
            cft0 = pp.tile([_V, _N], BF16)
            cft1 = pp.tile([_V, _N], BF16)
            rwt = pp.tile([_V, 2 * _TILES * _V], BF16)
            mng = pp.tile([_V, _TILES], F32)
            zbs = pp.tile([_V, _TILES], F32)
            cfts = [cft0, cft1]

            nc.sync.dma_start(rwt[:], rwt_d[:])
            nc.sync.dma_start(mng[:], mng_d[:])
            for p in range(_SS):
                for kc in range(2):
                    nc.sync.dma_start(
                        cfts[kc][:, p * _SSW : (p + 1) * _SSW],
                        cft_d[kc * _V : (kc + 1) * _V, p * _SSW : (p + 1) * _SSW],
                    )

            for t in range(_TILES):
                et = ep.tile([_V, _N], BF16)
                pt = sp.tile([_V, 8], F32)
                zn = sp.tile([_V, 1], F32)
                w0, w1 = OFF[t], OFF[t] + WID[t]
                nseg = 0
                for ss in range(_SS):
                    lo, hi = ss * _SSW, (ss + 1) * _SSW
                    acc = qp.tile([_V, _SSW], F32)
                    for kc in range(2):
                        lhsT = rwt[:, (2 * t + kc) * _V : (2 * t + kc + 1) * _V]
                        for cc in range(_SSW // _CW):
                            c0 = ss * _SSW + cc * _CW
                            nc.tensor.matmul(
                                acc[:, cc * _CW : (cc + 1) * _CW],
                                lhsT,
                                cfts[kc][:, c0 : c0 + _CW],
                                start=(kc == 0),
                                stop=(kc == 1),
                            )
                    cuts = sorted({lo, hi, min(max(w0, lo), hi), min(max(w1, lo), hi)})
                    for a, b in zip(cuts, cuts[1:]):
                        if a == b:
                            continue
                        inwin = a >= w0 and b <= w1
                        kw = {}
                        if not inwin:
                            kw["accum_out"] = pt[:, nseg : nseg + 1]
                            nseg += 1
                        nc.scalar.activation(
                            et[:, a:b],
                            acc[:, a - lo : b - lo],
                            mybir.ActivationFunctionType.Exp,
                            bias=mng[:, t : t + 1],
                            scale=1.0 / _T,
                            **kw,
                        )
                if nseg > 0:
                    nc.vector.tensor_reduce(
                        zn[:], pt[:, 0:nseg], mybir.AxisListType.X,
                        mybir.AluOpType.add,
                    )
                else:
                    nc.vector.memset(zn[:], 0.0)
                nc.vector.tensor_scalar(
                    zbs[:, t : t + 1], zn[:], 0.0, _EPS,
                    mybir.AluOpType.max, mybir.AluOpType.add,
                )
                nc.sync.dma_start(out_d[t * _V : (t + 1) * _V, 0:1], zbs[:, t : t + 1])
                lns = lp.tile([_V, WID[t]], BF16)
                lb = sp.tile([_V, 1], F32)
                nc.scalar.activation(
                    lns[:],
                    et[:, OFF[t] : OFF[t] + WID[t]],
                    mybir.ActivationFunctionType.Ln,
                    bias=zbs[:, t : t + 1],
                    accum_out=lb[:],
                )
                nc.sync.dma_start(out_d[t * _V : (t + 1) * _V, 1:2], lb[:])

    nc.finalize()
    return dict(nc=nc, aperm=aperm, slab=slab, OFF=OFF, WID=WID,
                first=first, last=last)


def kernel(feats_, labels_):
    global LAST_EXEC_NS
    from concourse.bass_utils import run_bass_kernel_spmd

    labels = np.asarray(labels_, dtype=np.int32)
    feats = np.asarray(feats_, dtype=np.float32)
    key = labels.tobytes()
    if key not in _cache:
        _cache[key] = _build(labels)
    pr = _cache[key]
    aperm, slab = pr["aperm"], pr["slab"]
    OFF, WID = pr["OFF"], pr["WID"]

    import ml_dtypes

    BF = ml_dtypes.bfloat16
    G = feats[aperm].reshape(_N, _D)
    G16 = G.astype(BF)
    cft = np.ascontiguousarray(G16.T)
    G64 = G.astype(np.float64)
    normsq = np.einsum("ij,ij->i", G64, G64)
    m = normsq / _T
    G16f = G16.astype(np.float64)
    m16 = np.einsum("ij,ij->i", G16f, G16f) / _T
    mneg32 = (-m16).astype(np.float32)

    in_maps = []
    for k in range(_NCORES):
        rwt = np.empty((_V, 2 * _TILES * _V), BF)
        mng = np.empty((_V, _TILES), np.float32)
        for t in range(_TILES):
            rb = (8 * t + k) * _V
            blk = G16[rb : rb + _V, :].T  # [256, 128]
            rwt[:, (2 * t) * _V : (2 * t + 1) * _V] = blk[:_V, :]
            rwt[:, (2 * t + 1) * _V : (2 * t + 2) * _V] = blk[_V:, :]
            mng[:, t] = mneg32[rb : rb + _V]
        in_maps.append({"cft": cft, "rwt": rwt, "mng": mng})

    res = run_bass_kernel_spmd(
        pr["nc"], in_maps, core_ids=list(range(_NCORES)), trace=PROFILE
    )
    LAST_EXEC_NS = res.exec_time_ns

    zb = np.empty(_N, np.float32)
    Lblk = np.empty(_N, np.float32)
    for k in range(_NCORES):
        o = res.results[k]["outv"]
        for t in range(_TILES):
            g0 = (8 * t + k) * _V
            zb[g0 : g0 + _V] = o[t * _V : (t + 1) * _V, 0]
            Lblk[g0 : g0 + _V] = o[t * _V : (t + 1) * _V, 1]

    row_slot = np.arange(_N) // _V
    counts = (pr["last"] - pr["first"]).astype(np.int64)     # n_c per slot
    ncls_rows = (counts[row_slot] * _V).astype(np.float64)   # same-class rows
    Wrow = np.asarray(WID, np.float64)[row_slot // 8]
    nwrong = Wrow - ncls_rows

    diag_term = np.log(np.float32(1.0) + zb).astype(np.float64)
    wrong_term = nwrong * np.log(zb.astype(np.float64))
    Lpos = Lblk.astype(np.float64) - diag_term - wrong_term

    row_class = slab[row_slot]
    Csum_row = np.empty((_N, _D), np.float64)
    for c in np.unique(slab):
        sel = row_class == c
        Csum_row[sel] = G64[sel].sum(axis=0)
    Scross = np.einsum("ij,ij->i", G64, Csum_row)
    S = (Scross - normsq) / _T - (ncls_rows - 1.0) * m
    P = ncls_rows - 1.0

    mlpp = (S - Lpos) / P
    loss = np.mean(-(_T / _BT) * mlpp)
    return np.array(loss, dtype=np.float32)


# revision 16
# speedup vs baseline: 2.4651x; 1.1103x over previous
import sys

sys.path.insert(0, "/opt/trn_rl_repo")
import numpy as np

_T = 0.1
_BT = 0.07
_EPS = 1e-10
_NCORES = 8
_A = 64        # anchors
_V = 128       # views per anchor
_D = 256       # feature dim
_N = _A * _V   # 8192 rows
_TILES = 8     # row tiles (128 rows) per core
_SS = 4        # supersteps per tile
_SSW = 2048    # superstep width (4 PSUM banks)
_CW = 512      # matmul moving width

PROFILE = False
LAST_EXEC_NS = None
_cache = {}


def _patch_act_tables():
    from concourse import bacc

    if getattr(bacc, "_joint_act_patch", False):
        return
    orig = bacc.get_activation_tables

    def _joint_only(arch):
        tabs = orig(arch)
        return {
            n: (s if n == "natural_log_exp_and_others" else set())
            for n, s in tabs.items()
        }

    bacc.get_activation_tables = _joint_only
    bacc._joint_act_patch = True


def _build(labels):
    from concourse import bass, bacc, mybir, tile

    _patch_act_tables()
    F32 = mybir.dt.float32
    BF16 = mybir.dt.bfloat16

    aperm = np.argsort(labels, kind="stable")
    slab = labels[aperm]
    first = np.searchsorted(slab, slab, side="left")
    last = np.searchsorted(slab, slab, side="right")
    OFF, WID = [], []
    for t in range(_TILES):
        lo = int(first[8 * t]) * _V
        hi = int(last[8 * t + 7]) * _V
        OFF.append(lo)
        WID.append(hi - lo)

    nc = bacc.Bacc()
    cft_d = nc.declare_dram_parameter("cft", [_D, _N], BF16, isOutput=False)
    rwt_d = nc.declare_dram_parameter("rwt", [_V, 2 * _TILES * _V], BF16, isOutput=False)
    mng_d = nc.declare_dram_parameter("mng", [_V, _TILES], F32, isOutput=False)
    out_d = nc.declare_dram_parameter("outv", [_TILES * _V, 2], F32, isOutput=True)

    with tile.TileContext(nc) as tc:
        with (
            tc.tile_pool(name="persist", bufs=1) as pp,
            tc.tile_pool(name="epool", bufs=3) as ep,
            tc.tile_pool(name="small", bufs=2) as sp,
            tc.tile_pool(name="lnp", bufs=2) as lp,
            tc.tile_pool(name="psum", bufs=2, space=bass.MemorySpace.PSUM) as qp,
        ):
            cft0 = pp.tile([_V, _N], BF16)
            cft1 = pp.tile([_V, _N], BF16)
            rwt = pp.tile([_V, 2 * _TILES * _V], BF16)
            mng = pp.tile([_V, _TILES], F32)
            ebs = pp.tile([_V, 1], F32)
            mxs = pp.tile([_V, _TILES], F32)
            cfts = [cft0, cft1]

            nc.vector.memset(ebs[:], _EPS)
            nc.sync.dma_start(rwt[:], rwt_d[:])
            nc.sync.dma_start(mng[:], mng_d[:])
            for p in range(_SS):
                for kc in range(2):
                    nc.sync.dma_start(
                        cfts[kc][:, p * _SSW : (p + 1) * _SSW],
                        cft_d[kc * _V : (kc + 1) * _V, p * _SSW : (p + 1) * _SSW],
                    )

            for t in range(_TILES):
                w0, w1 = OFF[t], OFF[t] + WID[t]
                wt = ep.tile([_V, WID[t]], BF16)
                mxp = sp.tile([_V, 8], F32)
                npiece = 0
                for ss in range(_SS):
                    lo, hi = ss * _SSW, (ss + 1) * _SSW
                    acc = qp.tile([_V, _SSW], F32)
                    for kc in range(2):
                        lhsT = rwt[:, (2 * t + kc) * _V : (2 * t + kc + 1) * _V]
                        for cc in range(_SSW // _CW):
                            c0 = ss * _SSW + cc * _CW
                            nc.tensor.matmul(
                                acc[:, cc * _CW : (cc + 1) * _CW],
                                lhsT,
                                cfts[kc][:, c0 : c0 + _CW],
                                start=(kc == 0),
                                stop=(kc == 1),
                            )
                    cuts = sorted({lo, hi, min(max(w0, lo), hi), min(max(w1, lo), hi)})
                    for a, b in zip(cuts, cuts[1:]):
                        if a == b:
                            continue
                        if a >= w0 and b <= w1:
                            nc.scalar.activation(
                                wt[:, a - w0 : b - w0],
                                acc[:, a - lo : b - lo],
                                mybir.ActivationFunctionType.Exp,
                                bias=mng[:, t : t + 1],
                                scale=1.0 / _T,
                            )
                        else:
                            nc.vector.tensor_reduce(
                                mxp[:, npiece : npiece + 1],
                                acc[:, a - lo : b - lo],
                                mybir.AxisListType.X,
                                mybir.AluOpType.max,
                            )
                            npiece += 1
                if npiece > 0:
                    nc.vector.tensor_reduce(
                        mxs[:, t : t + 1], mxp[:, 0:npiece],
                        mybir.AxisListType.X, mybir.AluOpType.max,
                    )
                else:
                    nc.vector.memset(mxs[:, t : t + 1], -3.0e38)
                nc.sync.dma_start(out_d[t * _V : (t + 1) * _V, 0:1], mxs[:, t : t + 1])
                lns = lp.tile([_V, WID[t]], BF16)
                lb = sp.tile([_V, 1], F32)
                nc.scalar.activation(
                    lns[:],
                    wt[:],
                    mybir.ActivationFunctionType.Ln,
                    bias=ebs[:, 0:1],
                    accum_out=lb[:],
                )
                nc.sync.dma_start(out_d[t * _V : (t + 1) * _V, 1:2], lb[:])

    nc.finalize()
    return dict(nc=nc, aperm=aperm, slab=slab, OFF=OFF, WID=WID,
                first=first, last=last)


def kernel(feats_, labels_):
    global LAST_EXEC_NS
    from concourse.bass_utils import run_bass_kernel_spmd

    labels = np.asarray(labels_, dtype=np.int32)
    feats = np.asarray(feats_, dtype=np.float32)
    key = labels.tobytes()
    if key not in _cache:
        _cache[key] = _build(labels)
    pr = _cache[key]
    aperm, slab = pr["aperm"], pr["slab"]
    OFF, WID = pr["OFF"], pr["WID"]

    import ml_dtypes

    BF = ml_dtypes.bfloat16
    G = feats[aperm].reshape(_N, _D)
    G16 = G.astype(BF)
    cft = np.ascontiguousarray(G16.T)
    G64 = G.astype(np.float64)
    normsq = np.einsum("ij,ij->i", G64, G64)
    m = normsq / _T
    G16f = G16.astype(np.float64)
    m16 = np.einsum("ij,ij->i", G16f, G16f) / _T
    mneg32 = (-m16).astype(np.float32)

    in_maps = []
    for k in range(_NCORES):
        rwt = np.empty((_V, 2 * _TILES * _V), BF)
        mng = np.empty((_V, _TILES), np.float32)
        for t in range(_TILES):
            rb = (8 * t + k) * _V
            blk = G16[rb : rb + _V, :].T  # [256, 128]
            rwt[:, (2 * t) * _V : (2 * t + 1) * _V] = blk[:_V, :]
            rwt[:, (2 * t + 1) * _V : (2 * t + 2) * _V] = blk[_V:, :]
            mng[:, t] = mneg32[rb : rb + _V]
        in_maps.append({"cft": cft, "rwt": rwt, "mng": mng})

    res = run_bass_kernel_spmd(
        pr["nc"], in_maps, core_ids=list(range(_NCORES)), trace=PROFILE
    )
    LAST_EXEC_NS = res.exec_time_ns

    mxr = np.empty(_N, np.float32)
    Lblk = np.empty(_N, np.float32)
    for k in range(_NCORES):
        o = res.results[k]["outv"]
        for t in range(_TILES):
            g0 = (8 * t + k) * _V
            mxr[g0 : g0 + _V] = o[t * _V : (t + 1) * _V, 0]
            Lblk[g0 : g0 + _V] = o[t * _V : (t + 1) * _V, 1]

    # out-of-window logits must underflow exp to exact f32 zero
    lim = _T * (m16 - 120.0)
    if not np.all(mxr.astype(np.float64) < lim):
        raise RuntimeError("out-of-window logit above underflow margin")

    row_slot = np.arange(_N) // _V
    counts = (pr["last"] - pr["first"]).astype(np.int64)     # n_c per slot
    ncls_rows = (counts[row_slot] * _V).astype(np.float64)   # same-class rows
    Wrow = np.asarray(WID, np.float64)[row_slot // 8]
    nwrong = Wrow - ncls_rows

    zbf = np.float32(_EPS)
    diag_term = np.float64(np.log(np.float32(1.0) + zbf))
    wrong_term = nwrong * np.log(np.float64(zbf))
    Lpos = Lblk.astype(np.float64) - diag_term - wrong_term

    row_class = slab[row_slot]
    Csum_row = np.empty((_N, _D), np.float64)
    for c in np.unique(slab):
        sel = row_class == c
        Csum_row[sel] = G64[sel].sum(axis=0)
    Scross = np.einsum("ij,ij->i", G64, Csum_row)
    S = (Scross - normsq) / _T - (ncls_rows - 1.0) * m
    P = ncls_rows - 1.0

    mlpp = (S - Lpos) / P
    loss = np.mean(-(_T / _BT) * mlpp)
    return np.array(loss, dtype=np.float32)
